# revision 43
# baseline (speedup 1.0000x reference)
"""Barrier_Net TRN2 kernel: 8-core data-parallel Bass/Tile implementation (v2).

Structure (all math on device; host does layout/packing only):
  - x sharded over 8 cores by agent axis (12500 each, padded to 13312 = 13*1024).
  - XT: feature-major transpose of x, fp16 [128, A] (rows 0..84 = features).
  - Deepset trick: phi_w2 is shared across neighbors, so the neighbor sum
    moves BEFORE the second matmul: phi2+rho1 collapse into ONE matmul on
    sum_n relu(z_n).  W1 runs as 12 block-diagonal matmuls per 512 agents
    (2 elements x 64 hidden = full 128-wide output), relu evacuations are
    split across Scalar/Vector/Pool engines, the neighbor sum is a small
    fp16 2x add tree on DVE, and W2R = (phi_w2 @ rho_w1) follows.
  - rho2+psi1 fold into W_rp = rho_w2 @ psi_w1[0:2]; the g-term uses a
    128-row padded weight against the full XT tile.
  - Barrier in agent-mod-128 layout [128, (bh, xy, n)], fp32 chain
    (square/sqrt/recip), reduced over n by DVE tensor_reduce.
  - e is transposed into agent-mod-128 layout by tiny PE transposes; both
    tanh stages run once per core at the end => only 2 activation-table
    loads in the whole kernel (sqrt set during the loop, tanh set at end).
"""
import sys
sys.path.insert(0, "/opt/trn_rl_repo")
import numpy as np
import concourse.bacc as bacc
import concourse.tile as tile
import concourse.mybir as mybir
from concourse.bass_utils import run_bass_kernel_spmd
from contextlib import ExitStack

F32 = mybir.dt.float32
F16 = mybir.dt.float16
AF = mybir.ActivationFunctionType
ALU = mybir.AluOpType

B, NN, NO, SD = 100000, 16, 8, 4
H, PHI_OUT, ADIM = 64, 16, 2
DS, B_GAMMA = 0.2, 0.01
D_OBS = 85
NCORE = 8
AC = B // NCORE            # 12500 real agents per core
NPAIR = 13                 # pairs of 512-agent groups
AP_ = NPAIR * 1024         # padded agents per core = 13312
YW = NPAIR * 16            # output cols = 208


def _pack_weights(phi_w1, phi_b1, phi_w2, phi_b2, obs_w1, obs_b1, obs_w2, obs_b2,
                  rho_w1, rho_b1, rho_w2, rho_b2, psi_w1, psi_b1, psi_w2, psi_b2):
    # W1 block-diagonal: matmul q covers neighbors (2q, 2q+1); lhsT [128,128]
    W1P = np.zeros((8, 128, 128), np.float32)
    for q in range(8):
        for j in range(2):
            n = 2 * q + j
            W1P[q, 5 + 4 * n:5 + 4 * n + 4, 64 * j:64 * j + 64] = phi_w1
    OW1P = np.zeros((4, 128, 128), np.float32)
    for m in range(4):
        for j in range(2):
            o = 2 * m + j
            OW1P[m, 69 + 2 * o:69 + 2 * o + 2, 64 * j:64 * j + 64] = obs_w1
    # W2R fold: phi2 + rho1 (applied to post-relu neighbor sums)
    w2r_phi = (phi_w2 @ rho_w1).astype(np.float32)       # [64, 64]
    w2r_obs = (obs_w2 @ rho_w1).astype(np.float32)
    W2RP = np.zeros((128, 64), np.float32)
    W2RP[0:64] = w2r_phi
    W2RP[64:128] = w2r_phi
    W2RO = np.zeros((128, 64), np.float32)
    W2RO[0:64] = w2r_obs
    W2RO[64:128] = w2r_obs
    # rho2 + psi1 fold; duplicated on both partition halves (col-tiled heads)
    wrp = (rho_w2 @ psi_w1[0:2]).astype(np.float32)      # [64, 64]
    WRP2 = np.zeros((128, 64), np.float32)
    WRP2[0:64] = wrp
    WRP2[64:128] = wrp
    PW1B = np.zeros((128, 64), np.float32)
    PW1B[0] = psi_w1[2]                                   # g0 = x[:,0] (=NN)
    PW1B[1] = psi_w1[3]                                   # g1 = x[:,1]
    # e head as eT = psi_h^T @ psi_w2: psi_w2 on both partition halves
    PW2D = np.zeros((128, 2), np.float32)
    PW2D[0:64] = psi_w2
    PW2D[64:128] = psi_w2
    # biases
    small = np.zeros((128, 8), np.float32)
    small[:, 0] = np.tile(phi_b1, 2)
    small[:, 1] = np.tile(obs_b1, 2)
    rin_bias = NN * phi_b2 + NO * obs_b2
    small[:, 2] = np.tile(rho_b1 + rin_bias @ rho_w1, 2)
    small[:, 3] = np.tile(psi_b1 + rho_b2 @ psi_w1[0:2], 2)
    small[0:4, 4] = np.tile(psi_b2, 2)
    pb2t = np.tile(psi_b2, (128, YW // 2)).astype(np.float32)
    return dict(
        w1p=W1P.transpose(1, 0, 2).reshape(128, 8 * 128).astype(np.float16),
        ow1p=OW1P.transpose(1, 0, 2).reshape(128, 4 * 128).astype(np.float16),
        w2rp=W2RP.astype(np.float16), w2ro=W2RO.astype(np.float16),
        wrp2=WRP2.astype(np.float16), pw1b=PW1B.astype(np.float16),
        pw2d=PW2D.astype(np.float16),
        small=small, pb2t=pb2t)


def _build(nc):
    xt_d = nc.dram_tensor("xt", [128, AP_], F16, kind="ExternalInput").ap()
    xb_d = nc.dram_tensor("xb", [128, NPAIR * 256], F32, kind="ExternalInput").ap()
    w1p_d = nc.dram_tensor("w1p", [128, 8 * 128], F16, kind="ExternalInput").ap()
    ow1p_d = nc.dram_tensor("ow1p", [128, 4 * 128], F16, kind="ExternalInput").ap()
    w2rp_d = nc.dram_tensor("w2rp", [128, 64], F16, kind="ExternalInput").ap()
    w2ro_d = nc.dram_tensor("w2ro", [128, 64], F16, kind="ExternalInput").ap()
    wrp2_d = nc.dram_tensor("wrp2", [128, 64], F16, kind="ExternalInput").ap()
    pw1b_d = nc.dram_tensor("pw1b", [128, 64], F16, kind="ExternalInput").ap()
    pw2d_d = nc.dram_tensor("pw2d", [128, 2], F16, kind="ExternalInput").ap()
    small_d = nc.dram_tensor("small", [128, 8], F32, kind="ExternalInput").ap()
    pb2t_d = nc.dram_tensor("pb2t", [128, YW], F32, kind="ExternalInput").ap()
    y_d = nc.dram_tensor("y", [128, YW], F32, kind="ExternalOutput").ap()

    with tile.TileContext(nc) as tc, ExitStack() as ctx:
        cw = ctx.enter_context(tc.tile_pool(name="cw", bufs=1))
        xin = ctx.enter_context(tc.tile_pool(name="xin", bufs=3))
        xbin = ctx.enter_context(tc.tile_pool(name="xbin", bufs=3))
        sv = ctx.enter_context(tc.tile_pool(name="sv", bufs=4))
        tr = ctx.enter_context(tc.tile_pool(name="tr", bufs=4))
        hd = ctx.enter_context(tc.tile_pool(name="hd", bufs=3))
        br = ctx.enter_context(tc.tile_pool(name="br", bufs=3))
        keep = ctx.enter_context(tc.tile_pool(name="keep", bufs=1))
        p1 = ctx.enter_context(tc.tile_pool(name="p1", bufs=5, space="PSUM"))
        pr = ctx.enter_context(tc.tile_pool(name="pr", bufs=1, space="PSUM"))
        pp = ctx.enter_context(tc.tile_pool(name="pp", bufs=1, space="PSUM"))
        pt = ctx.enter_context(tc.tile_pool(name="pt", bufs=1, space="PSUM"))

        # pair-0 inputs first so the first W1 matmul isn't stuck behind
        # nine weight DMAs on the sync queue
        xt0 = xin.tile([128, 1024], F16, tag="xt", name="xt0")
        nc.sync.dma_start(xt0[:], xt_d[:, 0:1024])
        xb0 = xbin.tile([128, 256], F32, tag="xb", name="xb0")
        nc.sync.dma_start(xb0[:], xb_d[:, 0:256])
        w1p = cw.tile([128, 8 * 128], F16); nc.sync.dma_start(w1p[:], w1p_d)
        ow1p = cw.tile([128, 4 * 128], F16); nc.sync.dma_start(ow1p[:], ow1p_d)
        w2rp = cw.tile([128, 64], F16); nc.sync.dma_start(w2rp[:], w2rp_d)
        w2ro = cw.tile([128, 64], F16); nc.sync.dma_start(w2ro[:], w2ro_d)
        wrp2 = cw.tile([128, 64], F16); nc.sync.dma_start(wrp2[:], wrp2_d)
        pw1b = cw.tile([128, 64], F16); nc.sync.dma_start(pw1b[:], pw1b_d)
        pw2d = cw.tile([128, 2], F16); nc.sync.dma_start(pw2d[:], pw2d_d)
        smallt = cw.tile([128, 8], F32); nc.sync.dma_start(smallt[:], small_d)
        pb2t = cw.tile([128, YW], F32); nc.sync.dma_start(pb2t[:], pb2t_d)

        barc = keep.tile([128, YW], F32, tag="barc")       # barrier collect
        eT = pt.tile([128, YW], F32, tag="eT")             # transposed e collect

        def evac(kind, dst, src, bias):
            if kind == 0:
                nc.scalar.activation(dst, src, AF.Relu, bias=bias)
            else:
                nc.vector.tensor_scalar(dst, src, bias, 0.0,
                                        op0=ALU.add, op1=ALU.max)

        # evac engine per tile t (0..5): Act / DVE only (gpsimd has no PSUM port)
        EV = [0, 1, 0, 1, 0, 1]

        # --- pipelined stage helpers; heads lag one pair, transposes two ---
        def issue_rho_h(st):
            st["rho_h"] = hd.tile([128, 512], F16, tag="rho_h", name="rho_h")
            nc.scalar.activation(st["rho_h"][:], st["rho_ps"][:], AF.Relu,
                                 bias=smallt[:, 2:3])

        def issue_B(st):
            st["psi_ps"] = pp.tile([128, 512], F32, tag="psi", name="psi_ps")
            for g in range(2):
                gc = slice(g * 512, g * 512 + 512)
                po = st["psi_ps"][g * 64:g * 64 + 64, :]
                nc.tensor.matmul(po, lhsT=wrp2[g * 64:g * 64 + 64, :],
                                 rhs=st["rho_h"][g * 64:g * 64 + 64, :],
                                 start=True, stop=False)
                nc.tensor.matmul(po, lhsT=pw1b, rhs=st["xt"][:, gc],
                                 start=False, stop=True)

        def issue_psi_h(st):
            st["psi_h"] = hd.tile([128, 512], F16, tag="psi_h", name="psi_h")
            nc.scalar.activation(st["psi_h"][:], st["psi_ps"][:], AF.Relu,
                                 bias=smallt[:, 3:4])

        def issue_eT(st):
            # eT block = psi_h_block^T @ psi_w2 : e directly in transposed
            # (agent-mod-128) layout; no separate e matmul/evac/transpose.
            p, psi_h = st["p"], st["psi_h"]
            for g in range(2):
                hp = slice(g * 64, g * 64 + 64)
                for b in range(4):
                    c0 = p * 16 + 4 * b + 2 * g
                    nc.tensor.matmul(eT[:, c0:c0 + 2],
                                     lhsT=psi_h[hp, 128 * b:128 * b + 128],
                                     rhs=pw2d[hp, :], start=True, stop=True)

        def issue_barrier(st):
            # xb free layout = bh*32 + xy*16 + n
            p, xb = st["p"], st["xb"]
            xbr = xb[:].rearrange("q (bh xy n) -> q bh xy n", xy=2, n=16)
            sq = br.tile([128, 256], F32, tag="sq")
            nc.gpsimd.tensor_mul(sq[:], xb[:], xb[:])
            sqr = sq[:].rearrange("q (bh xy n) -> q bh xy n", xy=2, n=16)
            n2 = br.tile([128, 128], F32, tag="n2")
            n2r = n2[:].rearrange("q (bh n) -> q bh n", n=16)
            nc.gpsimd.tensor_add(n2r, sqr[:, :, 0], sqr[:, :, 1])
            rd = br.tile([128, 128], F32, tag="rd")
            nc.scalar.activation(rd[:], n2[:], AF.Sqrt)
            dd = br.tile([128, 128], F32, tag="dd")
            nc.gpsimd.tensor_scalar(dd[:], rd[:], -DS, 1.0 / B_GAMMA,
                                    op0=ALU.add, op1=ALU.mult)
            f = br.tile([128, 128], F32, tag="f")
            nc.vector.reciprocal_approx_fast(out=f[:], in_=dd[:])
            fr = f[:].rearrange("q (bh n) -> q bh n", n=16)
            fp = br.tile([128, 256], F16, tag="fp")
            fpr = fp[:].rearrange("q (bh xy n) -> q bh xy n", xy=2, n=16)
            nc.gpsimd.tensor_mul(fpr[:, :, 0], xbr[:, :, 0], fr)
            nc.gpsimd.tensor_mul(fpr[:, :, 1], xbr[:, :, 1], fr)
            nc.vector.tensor_reduce(
                out=barc[:, p * 16:p * 16 + 16].rearrange(
                    "q (bh xy) -> q bh xy", xy=2),
                in_=fpr, axis=mybir.AxisListType.X, op=ALU.add)

        def issue_group(st, g):
            gc = slice(g * 512, g * 512 + 512)
            xt = st["xt"]
            S = []
            t01 = t23 = None
            for t in range(6):
                s = sv.tile([128, 1024], F16, tag=f"s{t}")
                for h in range(2):
                    q = 2 * t + h
                    if q < 8:
                        la = w1p[:, 128 * q:128 * q + 128]
                    else:
                        la = ow1p[:, 128 * (q - 8):128 * (q - 8) + 128]
                    ps = p1.tile([128, 512], F32, tag="ps")
                    nc.tensor.matmul(ps[:], lhsT=la, rhs=xt[:, gc],
                                     start=True, stop=True)
                    bias = smallt[:, 0:1] if t < 4 else smallt[:, 1:2]
                    evac(EV[t], s[:, 512 * h:512 * h + 512], ps[:], bias)
                S.append(s)
                # interleave the tree so Pool starts as soon as inputs land
                if t == 1:
                    t01 = tr.tile([128, 1024], F16, tag="t01")
                    nc.gpsimd.tensor_add(t01[:], S[0][:], S[1][:])
                elif t == 3:
                    t23 = tr.tile([128, 1024], F16, tag="t23")
                    nc.gpsimd.tensor_add(t23[:], S[2][:], S[3][:])
            tp = tr.tile([128, 1024], F16, tag="tp")
            nc.gpsimd.tensor_add(tp[:], t01[:], t23[:])
            t45 = tr.tile([128, 1024], F16, tag="t45")
            nc.gpsimd.tensor_add(t45[:], S[4][:], S[5][:])
            st[f"tree{g}"] = (tp, t45)

        def issue_W2R(st, g):
            tp, t45 = st[f"tree{g}"]
            ro = st["rho_ps"][g * 64:g * 64 + 64, :]
            nc.tensor.matmul(ro, lhsT=w2rp, rhs=tp[:, 0:512],
                             start=True, stop=False)
            nc.tensor.matmul(ro, lhsT=w2rp, rhs=tp[:, 512:1024],
                             start=False, stop=False)
            nc.tensor.matmul(ro, lhsT=w2ro, rhs=t45[:, 0:512],
                             start=False, stop=False)
            nc.tensor.matmul(ro, lhsT=w2ro, rhs=t45[:, 512:1024],
                             start=False, stop=True)

        prev = None      # pair p-1 state (heads pending)
        prev2 = None     # pair p-2 state (transposes pending)
        for p in range(NPAIR):
            cs = p * 1024
            st = {"p": p}
            if p == 0:
                st["xt"], st["xb"] = xt0, xb0
            else:
                st["xt"] = xin.tile([128, 1024], F16, tag="xt", name="xt")
                nc.sync.dma_start(st["xt"][:], xt_d[:, cs:cs + 1024])
                st["xb"] = xbin.tile([128, 256], F32, tag="xb", name="xb")
                nc.sync.dma_start(st["xb"][:], xb_d[:, p * 256:p * 256 + 256])

            st["rho_ps"] = pr.tile([128, 512], F32, tag="rho", name="rho_ps")
            issue_group(st, 0)
            if prev is not None:
                issue_W2R(prev, 1)
                issue_rho_h(prev)
                issue_B(prev)
            issue_barrier(st)
            if prev is not None:
                issue_psi_h(prev)
            issue_group(st, 1)
            if prev is not None:
                issue_eT(prev)
            issue_W2R(st, 0)
            prev2, prev = prev, st

        # drain the pipeline
        issue_W2R(prev, 1)
        issue_rho_h(prev)
        issue_B(prev)
        issue_psi_h(prev)
        issue_eT(prev)

        # ---- final phase: empty = tanh(eT + pb2); y = 2*tanh(empty + barrier)
        pre1 = keep.tile([128, YW], F32, tag="pre1")
        nc.vector.tensor_add(pre1[:], eT[:], pb2t[:])
        emp = keep.tile([128, YW], F16, tag="emp")
        nc.scalar.activation(emp[:], pre1[:], AF.Tanh)
        pre = keep.tile([128, YW], F32, tag="pre")
        nc.gpsimd.tensor_add(pre[:], emp[:], barc[:])
        act2 = keep.tile([128, YW], F32, tag="act2")
        nc.scalar.activation(act2[:], pre[:], AF.Tanh)
        yt = keep.tile([128, YW], F32, tag="yt")
        nc.gpsimd.tensor_scalar_mul(yt[:], act2[:], 2.0)
        nc.sync.dma_start(y_d, yt[:])
    return nc


def _pack_inputs(x):
    """Per-core input maps. x: [B, 85] fp32."""
    maps = []
    for c in range(NCORE):
        xs = x[c * AC:(c + 1) * AC]
        xp = np.zeros((AP_, D_OBS), np.float32)
        xp[:AC] = xs
        xt = np.zeros((128, AP_), np.float16)
        xt[0:D_OBS] = xp.T.astype(np.float16)
        # barrier operand: [r, p*256 + bh*32 + xy*16 + n] = -x[a, 5+4n+xy]
        # a = p*1024 + h*512 + b*128 + r ; bh = 2*b + h
        nb = -xp[:, 5:69].reshape(AP_, 16, 4)[:, :, 0:2]    # [A, n, xy]
        v = nb.reshape(NPAIR, 2, 4, 128, 16, 2)             # p h b r n xy
        v = v.transpose(3, 0, 2, 1, 5, 4)                   # r p b h xy n
        xb = np.ascontiguousarray(
            v.reshape(128, NPAIR, 2, 4, 2, 16).reshape(128, NPAIR * 256),
            dtype=np.float32)
        maps.append({"xt": np.ascontiguousarray(xt), "xb": xb})
    return maps


def _decode(Y):
    """Y: [128, YW] -> [AP_, 2]. col = p*16 + 4b + 2h + xy."""
    v = Y.reshape(128, NPAIR, 4, 2, 2)                      # r p b h xy
    return v.transpose(1, 3, 2, 0, 4).reshape(AP_, 2)       # p h b r xy


_CACHED = {}


def kernel(**inputs):
    x = np.asarray(inputs["x"], np.float32)
    wk = _pack_weights(**{k: np.asarray(v, np.float32) for k, v in inputs.items()
                          if k != "x"})
    in_maps = _pack_inputs(x)
    for m in in_maps:
        m.update(wk)

    if "nc" not in _CACHED:
        nc = bacc.Bacc("TRN2", target_bir_lowering=False, debug=False,
                       num_devices=NCORE)
        _build(nc)
        nc.compile()
        _CACHED["nc"] = nc
    nc = _CACHED["nc"]
    res = run_bass_kernel_spmd(nc, in_maps, core_ids=list(range(NCORE)))
    out = np.empty((B, ADIM), np.float32)
    for c in range(NCORE):
        out[c * AC:(c + 1) * AC] = _decode(res.results[c]["y"])[:AC]
    return out


if __name__ == "__main__":
    import reference
    ins = {k: np.asarray(v) for k, v in reference.setup_inputs().items()}
    got = kernel(**ins)
    exp = np.asarray(reference.reference(**ins))
    err = np.abs(got - exp).max()
    rel = err / np.abs(exp).max()
    print(f"absmax {err:.4e} rel {rel:.4e}")


# revision 44
# speedup vs baseline: 1.0121x; 1.0121x over previous
"""Barrier_Net TRN2 kernel: 8-core data-parallel Bass/Tile implementation (v2).

Structure (all math on device; host does layout/packing only):
  - x sharded over 8 cores by agent axis (12500 each, padded to 13312 = 13*1024).
  - XT: feature-major transpose of x, fp16 [128, A] (rows 0..84 = features).
  - Deepset trick: phi_w2 is shared across neighbors, so the neighbor sum
    moves BEFORE the second matmul: phi2+rho1 collapse into ONE matmul on
    sum_n relu(z_n).  W1 runs as 12 block-diagonal matmuls per 512 agents
    (2 elements x 64 hidden = full 128-wide output), relu evacuations are
    split across Scalar/Vector/Pool engines, the neighbor sum is a small
    fp16 2x add tree on DVE, and W2R = (phi_w2 @ rho_w1) follows.
  - rho2+psi1 fold into W_rp = rho_w2 @ psi_w1[0:2]; the g-term uses a
    128-row padded weight against the full XT tile.
  - Barrier in agent-mod-128 layout [128, (bh, xy, n)], fp32 chain
    (square/sqrt/recip), reduced over n by DVE tensor_reduce.
  - e is transposed into agent-mod-128 layout by tiny PE transposes; both
    tanh stages run once per core at the end => only 2 activation-table
    loads in the whole kernel (sqrt set during the loop, tanh set at end).
"""
import sys
sys.path.insert(0, "/opt/trn_rl_repo")
import numpy as np
import concourse.bacc as bacc
import concourse.tile as tile
import concourse.mybir as mybir
from concourse.bass_utils import run_bass_kernel_spmd
from contextlib import ExitStack

F32 = mybir.dt.float32
F16 = mybir.dt.float16
AF = mybir.ActivationFunctionType
ALU = mybir.AluOpType

B, NN, NO, SD = 100000, 16, 8, 4
H, PHI_OUT, ADIM = 64, 16, 2
DS, B_GAMMA = 0.2, 0.01
D_OBS = 85
NCORE = 8
AC = B // NCORE            # 12500 real agents per core
NPAIR = 13                 # pairs of 512-agent groups
AP_ = NPAIR * 1024         # padded agents per core = 13312
YW = NPAIR * 16            # output cols = 208


def _pack_weights(phi_w1, phi_b1, phi_w2, phi_b2, obs_w1, obs_b1, obs_w2, obs_b2,
                  rho_w1, rho_b1, rho_w2, rho_b2, psi_w1, psi_b1, psi_w2, psi_b2):
    # W1 block-diagonal: matmul q covers neighbors (2q, 2q+1); lhsT [128,128]
    W1P = np.zeros((8, 128, 128), np.float32)
    for q in range(8):
        for j in range(2):
            n = 2 * q + j
            W1P[q, 5 + 4 * n:5 + 4 * n + 4, 64 * j:64 * j + 64] = phi_w1
    OW1P = np.zeros((4, 128, 128), np.float32)
    for m in range(4):
        for j in range(2):
            o = 2 * m + j
            OW1P[m, 69 + 2 * o:69 + 2 * o + 2, 64 * j:64 * j + 64] = obs_w1
    # W2R fold: phi2 + rho1 (applied to post-relu neighbor sums)
    w2r_phi = (phi_w2 @ rho_w1).astype(np.float32)       # [64, 64]
    w2r_obs = (obs_w2 @ rho_w1).astype(np.float32)
    W2RP = np.zeros((128, 64), np.float32)
    W2RP[0:64] = w2r_phi
    W2RP[64:128] = w2r_phi
    W2RO = np.zeros((128, 64), np.float32)
    W2RO[0:64] = w2r_obs
    W2RO[64:128] = w2r_obs
    # rho2 + psi1 fold; duplicated on both partition halves (col-tiled heads)
    wrp = (rho_w2 @ psi_w1[0:2]).astype(np.float32)      # [64, 64]
    WRP2 = np.zeros((128, 64), np.float32)
    WRP2[0:64] = wrp
    WRP2[64:128] = wrp
    PW1B = np.zeros((128, 64), np.float32)
    PW1B[0] = psi_w1[2]                                   # g0 = x[:,0] (=NN)
    PW1B[1] = psi_w1[3]                                   # g1 = x[:,1]
    # e head as eT = psi_h^T @ psi_w2: psi_w2 on both partition halves
    PW2D = np.zeros((128, 2), np.float32)
    PW2D[0:64] = psi_w2
    PW2D[64:128] = psi_w2
    # biases
    small = np.zeros((128, 8), np.float32)
    small[:, 0] = np.tile(phi_b1, 2)
    small[:, 1] = np.tile(obs_b1, 2)
    rin_bias = NN * phi_b2 + NO * obs_b2
    small[:, 2] = np.tile(rho_b1 + rin_bias @ rho_w1, 2)
    small[:, 3] = np.tile(psi_b1 + rho_b2 @ psi_w1[0:2], 2)
    small[0:4, 4] = np.tile(psi_b2, 2)
    pb2t = np.tile(psi_b2, (128, YW // 2)).astype(np.float32)
    return dict(
        w1p=W1P.transpose(1, 0, 2).reshape(128, 8 * 128).astype(np.float16),
        ow1p=OW1P.transpose(1, 0, 2).reshape(128, 4 * 128).astype(np.float16),
        w2rp=W2RP.astype(np.float16), w2ro=W2RO.astype(np.float16),
        wrp2=WRP2.astype(np.float16), pw1b=PW1B.astype(np.float16),
        pw2d=PW2D.astype(np.float16),
        small=small, pb2t=pb2t)


def _build(nc):
    xt_d = nc.dram_tensor("xt", [128, AP_], F16, kind="ExternalInput").ap()
    xb_d = nc.dram_tensor("xb", [128, NPAIR * 256], F32, kind="ExternalInput").ap()
    w1p_d = nc.dram_tensor("w1p", [128, 8 * 128], F16, kind="ExternalInput").ap()
    ow1p_d = nc.dram_tensor("ow1p", [128, 4 * 128], F16, kind="ExternalInput").ap()
    w2rp_d = nc.dram_tensor("w2rp", [128, 64], F16, kind="ExternalInput").ap()
    w2ro_d = nc.dram_tensor("w2ro", [128, 64], F16, kind="ExternalInput").ap()
    wrp2_d = nc.dram_tensor("wrp2", [128, 64], F16, kind="ExternalInput").ap()
    pw1b_d = nc.dram_tensor("pw1b", [128, 64], F16, kind="ExternalInput").ap()
    pw2d_d = nc.dram_tensor("pw2d", [128, 2], F16, kind="ExternalInput").ap()
    small_d = nc.dram_tensor("small", [128, 8], F32, kind="ExternalInput").ap()
    pb2t_d = nc.dram_tensor("pb2t", [128, YW], F32, kind="ExternalInput").ap()
    y_d = nc.dram_tensor("y", [128, YW], F32, kind="ExternalOutput").ap()

    with tile.TileContext(nc) as tc, ExitStack() as ctx:
        cw = ctx.enter_context(tc.tile_pool(name="cw", bufs=1))
        xin = ctx.enter_context(tc.tile_pool(name="xin", bufs=3))
        xbin = ctx.enter_context(tc.tile_pool(name="xbin", bufs=3))
        sv = ctx.enter_context(tc.tile_pool(name="sv", bufs=4))
        tr = ctx.enter_context(tc.tile_pool(name="tr", bufs=4))
        hd = ctx.enter_context(tc.tile_pool(name="hd", bufs=3))
        br = ctx.enter_context(tc.tile_pool(name="br", bufs=3))
        keep = ctx.enter_context(tc.tile_pool(name="keep", bufs=1))
        p1 = ctx.enter_context(tc.tile_pool(name="p1", bufs=5, space="PSUM"))
        pr = ctx.enter_context(tc.tile_pool(name="pr", bufs=1, space="PSUM"))
        pp = ctx.enter_context(tc.tile_pool(name="pp", bufs=1, space="PSUM"))
        pt = ctx.enter_context(tc.tile_pool(name="pt", bufs=1, space="PSUM"))

        # pair-0 inputs first so the first W1 matmul isn't stuck behind
        # nine weight DMAs on the sync queue
        xt0 = xin.tile([128, 1024], F16, tag="xt", name="xt0")
        nc.sync.dma_start(xt0[:], xt_d[:, 0:1024])
        xb0 = xbin.tile([128, 256], F32, tag="xb", name="xb0")
        nc.sync.dma_start(xb0[:], xb_d[:, 0:256])
        w1p = cw.tile([128, 8 * 128], F16); nc.sync.dma_start(w1p[:], w1p_d)
        ow1p = cw.tile([128, 4 * 128], F16); nc.sync.dma_start(ow1p[:], ow1p_d)
        w2rp = cw.tile([128, 64], F16); nc.sync.dma_start(w2rp[:], w2rp_d)
        w2ro = cw.tile([128, 64], F16); nc.sync.dma_start(w2ro[:], w2ro_d)
        wrp2 = cw.tile([128, 64], F16); nc.sync.dma_start(wrp2[:], wrp2_d)
        pw1b = cw.tile([128, 64], F16); nc.sync.dma_start(pw1b[:], pw1b_d)
        pw2d = cw.tile([128, 2], F16); nc.sync.dma_start(pw2d[:], pw2d_d)
        smallt = cw.tile([128, 8], F32); nc.sync.dma_start(smallt[:], small_d)
        pb2t = cw.tile([128, YW], F32); nc.sync.dma_start(pb2t[:], pb2t_d)

        barc = keep.tile([128, YW], F32, tag="barc")       # barrier collect
        eT = pt.tile([128, YW], F32, tag="eT")             # transposed e collect

        def evac(kind, dst, src, bias):
            if kind == 0:
                nc.scalar.activation(dst, src, AF.Relu, bias=bias)
            else:
                nc.vector.tensor_scalar(dst, src, bias, 0.0,
                                        op0=ALU.add, op1=ALU.max)

        # evac engine per tile t (0..5): Act / DVE only (gpsimd has no PSUM port)
        EV = [0, 1, 0, 1, 0, 1]

        # --- pipelined stage helpers; heads lag one pair, transposes two ---
        def issue_rho_h(st):
            st["rho_h"] = hd.tile([128, 512], F16, tag="rho_h", name="rho_h")
            nc.scalar.activation(st["rho_h"][:], st["rho_ps"][:], AF.Relu,
                                 bias=smallt[:, 2:3])

        def issue_B(st):
            st["psi_ps"] = pp.tile([128, 512], F32, tag="psi", name="psi_ps")
            for g in range(2):
                gc = slice(g * 512, g * 512 + 512)
                po = st["psi_ps"][g * 64:g * 64 + 64, :]
                nc.tensor.matmul(po, lhsT=wrp2[g * 64:g * 64 + 64, :],
                                 rhs=st["rho_h"][g * 64:g * 64 + 64, :],
                                 start=True, stop=False)
                nc.tensor.matmul(po, lhsT=pw1b, rhs=st["xt"][:, gc],
                                 start=False, stop=True)

        def issue_psi_h(st):
            st["psi_h"] = hd.tile([128, 512], F16, tag="psi_h", name="psi_h")
            nc.scalar.activation(st["psi_h"][:], st["psi_ps"][:], AF.Relu,
                                 bias=smallt[:, 3:4])

        def issue_eT(st):
            # eT block = psi_h_block^T @ psi_w2 : e directly in transposed
            # (agent-mod-128) layout; no separate e matmul/evac/transpose.
            p, psi_h = st["p"], st["psi_h"]
            for g in range(2):
                hp = slice(g * 64, g * 64 + 64)
                for b in range(4):
                    c0 = p * 16 + 4 * b + 2 * g
                    nc.tensor.matmul(eT[:, c0:c0 + 2],
                                     lhsT=psi_h[hp, 128 * b:128 * b + 128],
                                     rhs=pw2d[hp, :], start=True, stop=True)

        def issue_barrier(st):
            # xb free layout = bh*32 + xy*16 + n
            p, xb = st["p"], st["xb"]
            xbr = xb[:].rearrange("q (bh xy n) -> q bh xy n", xy=2, n=16)
            sq = br.tile([128, 256], F32, tag="sq")
            nc.gpsimd.tensor_mul(sq[:], xb[:], xb[:])
            sqr = sq[:].rearrange("q (bh xy n) -> q bh xy n", xy=2, n=16)
            n2 = br.tile([128, 128], F32, tag="n2")
            n2r = n2[:].rearrange("q (bh n) -> q bh n", n=16)
            nc.gpsimd.tensor_add(n2r, sqr[:, :, 0], sqr[:, :, 1])
            rd = br.tile([128, 128], F32, tag="rd")
            nc.scalar.activation(rd[:], n2[:], AF.Sqrt)
            dd = br.tile([128, 128], F32, tag="dd")
            nc.gpsimd.tensor_scalar(dd[:], rd[:], -DS, 1.0 / B_GAMMA,
                                    op0=ALU.add, op1=ALU.mult)
            f = br.tile([128, 128], F32, tag="f")
            nc.vector.reciprocal_approx_fast(out=f[:], in_=dd[:])
            fr = f[:].rearrange("q (bh n) -> q bh n", n=16)
            fp = br.tile([128, 256], F16, tag="fp")
            fpr = fp[:].rearrange("q (bh xy n) -> q bh xy n", xy=2, n=16)
            nc.gpsimd.tensor_mul(fpr[:, :, 0], xbr[:, :, 0], fr)
            nc.gpsimd.tensor_mul(fpr[:, :, 1], xbr[:, :, 1], fr)
            nc.vector.tensor_reduce(
                out=barc[:, p * 16:p * 16 + 16].rearrange(
                    "q (bh xy) -> q bh xy", xy=2),
                in_=fpr, axis=mybir.AxisListType.X, op=ALU.add)

        def issue_group(st, g):
            gc = slice(g * 512, g * 512 + 512)
            xt = st["xt"]
            S = []
            t01 = t23 = None
            for t in range(6):
                s = sv.tile([128, 1024], F16, tag=f"s{t}")
                for h in range(2):
                    q = 2 * t + h
                    if q < 8:
                        la = w1p[:, 128 * q:128 * q + 128]
                    else:
                        la = ow1p[:, 128 * (q - 8):128 * (q - 8) + 128]
                    ps = p1.tile([128, 512], F32, tag="ps")
                    nc.tensor.matmul(ps[:], lhsT=la, rhs=xt[:, gc],
                                     start=True, stop=True)
                    bias = smallt[:, 0:1] if t < 4 else smallt[:, 1:2]
                    kind = EV[t]
                    # fractional rebalance: shift one half-evac Act->DVE
                    # every third pair to equalize engine load
                    if kind == 0 and t == 0 and h == 0 and g == 0 \
                            and st["p"] % 3 == 0:
                        kind = 1
                    evac(kind, s[:, 512 * h:512 * h + 512], ps[:], bias)
                S.append(s)
                # interleave the tree so Pool starts as soon as inputs land
                if t == 1:
                    t01 = tr.tile([128, 1024], F16, tag="t01")
                    nc.gpsimd.tensor_add(t01[:], S[0][:], S[1][:])
                elif t == 3:
                    t23 = tr.tile([128, 1024], F16, tag="t23")
                    nc.gpsimd.tensor_add(t23[:], S[2][:], S[3][:])
            tp = tr.tile([128, 1024], F16, tag="tp")
            nc.gpsimd.tensor_add(tp[:], t01[:], t23[:])
            t45 = tr.tile([128, 1024], F16, tag="t45")
            nc.gpsimd.tensor_add(t45[:], S[4][:], S[5][:])
            st[f"tree{g}"] = (tp, t45)

        def issue_W2R(st, g):
            tp, t45 = st[f"tree{g}"]
            ro = st["rho_ps"][g * 64:g * 64 + 64, :]
            nc.tensor.matmul(ro, lhsT=w2rp, rhs=tp[:, 0:512],
                             start=True, stop=False)
            nc.tensor.matmul(ro, lhsT=w2rp, rhs=tp[:, 512:1024],
                             start=False, stop=False)
            nc.tensor.matmul(ro, lhsT=w2ro, rhs=t45[:, 0:512],
                             start=False, stop=False)
            nc.tensor.matmul(ro, lhsT=w2ro, rhs=t45[:, 512:1024],
                             start=False, stop=True)

        prev = None      # pair p-1 state (heads pending)
        prev2 = None     # pair p-2 state (transposes pending)
        for p in range(NPAIR):
            cs = p * 1024
            st = {"p": p}
            if p == 0:
                st["xt"], st["xb"] = xt0, xb0
            else:
                st["xt"] = xin.tile([128, 1024], F16, tag="xt", name="xt")
                nc.sync.dma_start(st["xt"][:], xt_d[:, cs:cs + 1024])
                st["xb"] = xbin.tile([128, 256], F32, tag="xb", name="xb")
                nc.sync.dma_start(st["xb"][:], xb_d[:, p * 256:p * 256 + 256])

            st["rho_ps"] = pr.tile([128, 512], F32, tag="rho", name="rho_ps")
            issue_group(st, 0)
            if prev is not None:
                issue_W2R(prev, 1)
                issue_rho_h(prev)
                issue_B(prev)
            issue_barrier(st)
            if prev is not None:
                issue_psi_h(prev)
            issue_group(st, 1)
            if prev is not None:
                issue_eT(prev)
            issue_W2R(st, 0)
            prev2, prev = prev, st

        # drain the pipeline
        issue_W2R(prev, 1)
        issue_rho_h(prev)
        issue_B(prev)
        issue_psi_h(prev)
        issue_eT(prev)

        # ---- final phase: empty = tanh(eT + pb2); y = 2*tanh(empty + barrier)
        pre1 = keep.tile([128, YW], F32, tag="pre1")
        nc.vector.tensor_add(pre1[:], eT[:], pb2t[:])
        emp = keep.tile([128, YW], F16, tag="emp")
        nc.scalar.activation(emp[:], pre1[:], AF.Tanh)
        pre = keep.tile([128, YW], F32, tag="pre")
        nc.gpsimd.tensor_add(pre[:], emp[:], barc[:])
        act2 = keep.tile([128, YW], F32, tag="act2")
        nc.scalar.activation(act2[:], pre[:], AF.Tanh)
        yt = keep.tile([128, YW], F32, tag="yt")
        nc.gpsimd.tensor_scalar_mul(yt[:], act2[:], 2.0)
        nc.sync.dma_start(y_d, yt[:])
    return nc


def _pack_inputs(x):
    """Per-core input maps. x: [B, 85] fp32."""
    maps = []
    for c in range(NCORE):
        xs = x[c * AC:(c + 1) * AC]
        xp = np.zeros((AP_, D_OBS), np.float32)
        xp[:AC] = xs
        xt = np.zeros((128, AP_), np.float16)
        xt[0:D_OBS] = xp.T.astype(np.float16)
        # barrier operand: [r, p*256 + bh*32 + xy*16 + n] = -x[a, 5+4n+xy]
        # a = p*1024 + h*512 + b*128 + r ; bh = 2*b + h
        nb = -xp[:, 5:69].reshape(AP_, 16, 4)[:, :, 0:2]    # [A, n, xy]
        v = nb.reshape(NPAIR, 2, 4, 128, 16, 2)             # p h b r n xy
        v = v.transpose(3, 0, 2, 1, 5, 4)                   # r p b h xy n
        xb = np.ascontiguousarray(
            v.reshape(128, NPAIR, 2, 4, 2, 16).reshape(128, NPAIR * 256),
            dtype=np.float32)
        maps.append({"xt": np.ascontiguousarray(xt), "xb": xb})
    return maps


def _decode(Y):
    """Y: [128, YW] -> [AP_, 2]. col = p*16 + 4b + 2h + xy."""
    v = Y.reshape(128, NPAIR, 4, 2, 2)                      # r p b h xy
    return v.transpose(1, 3, 2, 0, 4).reshape(AP_, 2)       # p h b r xy


_CACHED = {}


def kernel(**inputs):
    x = np.asarray(inputs["x"], np.float32)
    wk = _pack_weights(**{k: np.asarray(v, np.float32) for k, v in inputs.items()
                          if k != "x"})
    in_maps = _pack_inputs(x)
    for m in in_maps:
        m.update(wk)

    if "nc" not in _CACHED:
        nc = bacc.Bacc("TRN2", target_bir_lowering=False, debug=False,
                       num_devices=NCORE)
        _build(nc)
        nc.compile()
        _CACHED["nc"] = nc
    nc = _CACHED["nc"]
    res = run_bass_kernel_spmd(nc, in_maps, core_ids=list(range(NCORE)))
    out = np.empty((B, ADIM), np.float32)
    for c in range(NCORE):
        out[c * AC:(c + 1) * AC] = _decode(res.results[c]["y"])[:AC]
    return out


if __name__ == "__main__":
    import reference
    ins = {k: np.asarray(v) for k, v in reference.setup_inputs().items()}
    got = kernel(**ins)
    exp = np.asarray(reference.reference(**ins))
    err = np.abs(got - exp).max()
    rel = err / np.abs(exp).max()
    print(f"absmax {err:.4e} rel {rel:.4e}")
